# revision 12
# baseline (speedup 1.0000x reference)
"""Trainium2 Bass kernel for nn_LocalEnergyCore (sampling / local energy MLP).

Contract: kernel(**inputs) takes FULL unsharded inputs, returns FULL output
(scalar f32). Internally shards z along batch across 8 NeuronCores.

Per-core device program (B_loc = 512 samples):
  - Host pre-gathers each site's 3x3xK neighborhood into packed fp8 chunk
    tensors ctx [80, n_sites, 512]: partition r = ctx entry (position-major,
    bit-minor, as in the reference), +1 ones row (folds b1 into the matmul),
    + zero pad rows to 80 partitions (DMAs whose partition count is a
    multiple of 16 spread across all 16 SDMA engines; 72-74 pin to 1-2
    engines). The dropped center-self entry is handled by zeroing that row
    of the per-variant weight matrix.
  - L1: 25 fp8 DoubleRow matmuls, one per site PAIR. lhsT [74, 2, 128] is
    block-diagonal ([W_a | 0], [0 | W_b]) so psum [128, 512] holds both
    sites' hidden pre-activations (h on partitions 0-63 / 64-127).
  - Relu + fp32->fp8 cast evacuates psum to SBUF [128, 512] tiles,
    alternating between the ACT and DVE engines.
  - L2: 25 plain fp8 matmuls accumulate logits into ONE [50, 512] psum
    tile. All 25 stationaries are sliding windows of a single [128, 98]
    tensor with the W2 column strip at cols 48/49 (pair p reads cols
    [48-2p, 98-2p), placing its two sites at out partitions 2p, 2p+1).
  - One DVE scalar_tensor_tensor: ((logit > -b2) != target) summed over
    batch -> counts [50, 1]; ones-matmul reduces over partitions; ACT
    scales by 1/(B*S). Host sums the 8 per-core partial means.
"""

import sys

for _p in ("/opt/trn_rl_repo",):
    if _p not in sys.path:
        sys.path.insert(0, _p)

import numpy as np
import ml_dtypes

B, K, H, W = 4096, 8, 64, 64
S, HID = 50, 64
NPAIR = S // 2
R = 80                 # 72 ctx rows + ones row + 7 zero pad rows: DMAs
                       # with partition counts that are multiples of 16
                       # spread across all 16 SDMA engines; 72-74 do not
N_CORES = 8
B_LOC = B // N_CORES

# ctx DMA chunk boundaries, in pairs (first small so L1_0 starts early)
CHUNKS = [(0, 2), (2, 9), (9, 17), (17, 25)]

FP8 = ml_dtypes.float8_e4m3

LAST_RESULTS = None  # test harness introspection


def _host_prep(z, W1, b1, W2, b2, b_idx, i_idx, j_idx):
    """Shard + lay out inputs; returns (in_maps, -b2)."""
    z = np.asarray(z, dtype=np.float32)
    W1 = np.asarray(W1, dtype=np.float32)
    b1 = np.asarray(b1, dtype=np.float32)
    W2 = np.asarray(W2, dtype=np.float32)
    b_idx = np.asarray(b_idx).astype(np.int64)
    i_idx = np.asarray(i_idx).astype(np.int64)
    j_idx = np.asarray(j_idx).astype(np.int64)

    di = np.repeat(np.array([-1, 0, 1]), 3)
    dj = np.tile(np.array([-1, 0, 1]), 3)
    ni = (i_idx[:, None] + di[None, :]) % H          # [S, 9]
    nj = (j_idx[:, None] + dj[None, :]) % W          # [S, 9]

    # [B, K, S, 9] -> ctx entries position-major, bit-minor: [B, S, 72]
    neigh = z[:, :, ni, nj]
    ctx_full = np.transpose(neigh, (0, 2, 3, 1)).reshape(B, S, 9 * K)
    ctx8 = ctx_full.astype(FP8)

    # per-variant [74, HID] weight matrix: entry t row = W1[t - (t > drop)],
    # row drop = 0 (the removed center-self entry), row 72 = b1, row 73 = 0.
    t = np.arange(9 * K)
    WV = np.zeros((K, R, HID), dtype=np.float32)
    for v in range(K):
        drop = 4 * K + v
        src = t - (t > drop)
        WV[v, :9 * K] = W1[np.minimum(src, 9 * K - 2)]
        WV[v, drop] = 0.0
        WV[v, 9 * K] = b1
    WV8 = WV.astype(FP8)

    # L1 stationary: [74, 50, 128]; pair p half q feeds psum partitions
    # q*64:(q+1)*64 with site 2p+q's variant matrix (block-diagonal).
    w1_np = np.zeros((R, S, 2 * HID), dtype=FP8)
    for s in range(S):
        q = s % 2
        w1_np[:, s, q * HID:(q + 1) * HID] = WV8[b_idx[s]]

    # L2 DoubleRow stationary [128, 24, 128]: group g = pairs (2g, 2g+1) =
    # sites 4g..4g+3; half q covers pair 2g+q; cols padded to 128 (DR needs
    # col_grp == 0xf). Final odd pair uses a plain [128, 50] stationary.
    W28 = W2.astype(FP8)
    w2dr_np = np.zeros((2 * HID, 2 * (NPAIR // 2), 2 * HID), dtype=FP8)
    for g in range(NPAIR // 2):
        for q in range(2):
            w2dr_np[0:HID, 2 * g + q, 4 * g + 2 * q] = W28
            w2dr_np[HID:2 * HID, 2 * g + q, 4 * g + 2 * q + 1] = W28
    w2last_np = np.zeros((2 * HID, S), dtype=FP8)
    w2last_np[0:HID, S - 2] = W28
    w2last_np[HID:2 * HID, S - 1] = W28

    in_maps = []
    for c in range(N_CORES):
        bs = slice(c * B_LOC, (c + 1) * B_LOC)
        # ctx [74, 50, 512]: [entry, site, batch]
        ctx_c = np.zeros((R, S, B_LOC), dtype=FP8)
        ctx_c[:9 * K] = ctx8[bs].transpose(2, 1, 0)
        ctx_c[9 * K] = np.float32(1.0)
        targ_c = np.ascontiguousarray(
            z[bs, b_idx, i_idx, j_idx].T)             # [50, 512] fp32
        m = {"w1a": np.ascontiguousarray(w1_np[:, 0:8, :]),
             "w1b": np.ascontiguousarray(w1_np[:, 8:S, :]),
             "w2dr": w2dr_np, "w2last": w2last_np, "targ": targ_c}
        for ci, (p0, p1) in enumerate(CHUNKS):
            m[f"ctx{ci}"] = np.ascontiguousarray(ctx_c[:, 2 * p0:2 * p1, :])
        in_maps.append(m)
    return in_maps, -float(np.asarray(b2))


def _build_program(neg_b2):
    """Emit the per-core Bass program (identical across cores)."""
    import concourse.bacc as bacc
    import concourse.mybir as mybir
    import concourse.tile as tile

    fp32 = mybir.dt.float32
    fp8 = mybir.dt.float8e4
    DR = mybir.MatmulPerfMode.DoubleRow

    nc = bacc.Bacc("TRN2", target_bir_lowering=False, debug=False,
                   num_devices=N_CORES)

    ctx_d = []
    for ci, (p0, p1) in enumerate(CHUNKS):
        ctx_d.append(nc.dram_tensor(f"ctx{ci}", [R, 2 * (p1 - p0), B_LOC],
                                    fp8, kind="ExternalInput"))
    w1a_d = nc.dram_tensor("w1a", [R, 8, 2 * HID], fp8, kind="ExternalInput")
    w1b_d = nc.dram_tensor("w1b", [R, S - 8, 2 * HID], fp8,
                           kind="ExternalInput")
    w2dr_d = nc.dram_tensor("w2dr", [2 * HID, 2 * (NPAIR // 2), 2 * HID],
                            fp8, kind="ExternalInput")
    w2last_d = nc.dram_tensor("w2last", [2 * HID, S], fp8,
                              kind="ExternalInput")
    targ_d = nc.dram_tensor("targ", [S, B_LOC], fp32, kind="ExternalInput")
    outp = nc.dram_tensor("out", [S, 1], fp32, kind="ExternalOutput")

    with tile.TileContext(nc) as tc:
        with (
            tc.tile_pool(name="const", bufs=1) as cpool,
            tc.tile_pool(name="hsb", bufs=8) as hpool,
            tc.tile_pool(name="ps", bufs=6, space="PSUM") as pspool,
            tc.tile_pool(name="psl", bufs=1, space="PSUM") as pslpool,
        ):
            # --- input DMAs -------------------------------------------
            # gpsimd (SWDGE): ctx chunks; sync/scalar HWDGE: the rest
            # (their descriptor generation runs on separate sequencers).
            ctx_t = []
            for ci, (p0, p1) in enumerate(CHUNKS):
                ct = cpool.tile([R, 2 * (p1 - p0), B_LOC], fp8,
                                tag=f"ctx{ci}", name=f"ctx_sb{ci}")
                ctx_t.append((ct, p0))
                nc.gpsimd.dma_start(out=ct[:, :, :].opt(), in_=ctx_d[ci][:, :, :].opt())

            w1_sb = cpool.tile([R, S, 2 * HID], fp8, tag="w1")
            nc.sync.dma_start(out=w1_sb[:, 0:8, :].opt(),
                              in_=w1a_d[:, :, :].opt())
            nc.sync.dma_start(out=w1_sb[:, 8:S, :].opt(),
                              in_=w1b_d[:, :, :].opt())
            w2dr_sb = cpool.tile([2 * HID, 2 * (NPAIR // 2), 2 * HID], fp8,
                                 tag="w2dr")
            nc.scalar.dma_start(out=w2dr_sb[:, :, :].opt(),
                                in_=w2dr_d[:, :, :].opt())
            w2last_sb = cpool.tile([2 * HID, S], fp8, tag="w2last")
            nc.scalar.dma_start(out=w2last_sb[:, :], in_=w2last_d[:, :])
            targ_sb = cpool.tile([S, B_LOC], fp32, tag="targ")
            nc.scalar.dma_start(out=targ_sb[:, :], in_=targ_d[:, :])

            logit_ps = pslpool.tile([2 * HID, B_LOC], fp32, tag="logit")

            # PE p-state warm-up: the tensor engine needs several us of
            # continuous work to reach full clock. Run dummy matmuls on a
            # zeroed tile (spare psum bank) while input DMAs are in flight
            # so the real pipeline starts at speed.
            warm_sb = cpool.tile([2 * HID, B_LOC], fp8, tag="warm")
            nc.vector.memset(warm_sb[:, :], 0.0)
            warm_ps = pslpool.tile([2 * HID, B_LOC], fp32, tag="warm_ps")
            for _ in range(11):
                nc.tensor.matmul(warm_ps[:, :], warm_sb[:, 0:2 * HID],
                                 warm_sb[:, :], start=True, stop=True)

            # --- pair pipeline ----------------------------------------
            def pair_rhs(p):
                for ct, p0 in reversed(ctx_t):
                    if p >= p0:
                        i = p - p0
                        return ct[:, 2 * i:2 * i + 2, :]
                raise AssertionError

            h_sb = {}

            def emit_l1(p):
                h_ps = pspool.tile([2 * HID, B_LOC], fp32, tag="hps",
                                   name=f"hps{p}")
                nc.tensor.matmul(
                    h_ps[:, :],
                    w1_sb[:, 2 * p:2 * p + 2, :],
                    pair_rhs(p),
                    start=True, stop=True, perf_mode=DR)
                g = p // 2
                if g not in h_sb:
                    h_sb[g] = hpool.tile([2 * HID, 2, B_LOC], fp8,
                                         tag="hsb", name=f"hsb{g}")
                if p % 2 == 0:
                    nc.scalar.activation(
                        out=h_sb[g][:, 0, :], in_=h_ps[:, :],
                        func=mybir.ActivationFunctionType.Relu,
                        bias=0.0, scale=1.0)
                else:
                    nc.vector.tensor_scalar_max(h_sb[g][:, 1, :],
                                                h_ps[:, :], 0.0)

            def emit_l2(g):
                if 2 * g + 1 < NPAIR:
                    nc.tensor.matmul(
                        logit_ps[:, :],
                        w2dr_sb[:, 2 * g:2 * g + 2, :],
                        h_sb[g][:, :, :],
                        start=(g == 0), stop=False, perf_mode=DR)
                else:
                    nc.tensor.matmul(
                        logit_ps[0:S, :],
                        w2last_sb[:, :],
                        h_sb[g][:, 0, :],
                        start=False, stop=True)

            # L2 for group g (pairs 2g, 2g+1 = sites 4g..4g+3) emitted two
            # pairs after its evacuations; psum pool (bufs=6) keeps L1 ahead.
            for p in range(NPAIR):
                emit_l1(p)
                if p >= 6 and p % 2 == 0:
                    emit_l2((p - 6) // 2)
            emit_l2(NPAIR // 2 - 2)
            emit_l2(NPAIR // 2 - 1)
            emit_l2(NPAIR // 2)

            # --- compare + reduce -------------------------------------
            junk = cpool.tile([S, B_LOC], fp32, tag="junk")
            counts = cpool.tile([S, 1], fp32, tag="counts")
            nc.vector.scalar_tensor_tensor(
                out=junk[:, :], in0=logit_ps[0:S, :], scalar=neg_b2,
                in1=targ_sb[:, :],
                op0=mybir.AluOpType.is_gt, op1=mybir.AluOpType.not_equal,
                accum_out=counts[:, :])

            nc.sync.dma_start(out=outp[:, :], in_=counts[:, :])

    nc.compile()
    return nc


def kernel(**inputs):
    global LAST_RESULTS
    from concourse.bass_utils import run_bass_kernel_spmd

    in_maps, neg_b2 = _host_prep(
        inputs["z"], inputs["W1"], inputs["b1"], inputs["W2"],
        inputs["b2"], inputs["b_idx"], inputs["i_idx"], inputs["j_idx"])

    nc = _build_program(neg_b2)

    res = run_bass_kernel_spmd(nc, in_maps, list(range(N_CORES)))
    LAST_RESULTS = res
    total = 0.0
    for r in res.results:
        total += float(np.asarray(r["out"], dtype=np.float64).sum())
    return np.float32(total / float(B * S))


# revision 13
# speedup vs baseline: 1.1061x; 1.1061x over previous
"""Trainium2 Bass kernel for nn_LocalEnergyCore (sampling / local energy MLP).

Contract: kernel(**inputs) takes FULL unsharded inputs, returns FULL output
(scalar f32). Internally shards z along batch across 8 NeuronCores.

Per-core device program (B_loc = 512 samples):
  - Host pre-gathers each site's 3x3xK neighborhood into packed fp8 chunk
    tensors ctx [80, n_sites, 512]: partition r = ctx entry (position-major,
    bit-minor, as in the reference), +1 ones row (folds b1 into the matmul),
    + zero pad rows to 80 partitions (DMAs whose partition count is a
    multiple of 16 spread across all 16 SDMA engines; 72-74 pin to 1-2
    engines). The dropped center-self entry is handled by zeroing that row
    of the per-variant weight matrix.
  - L1: 25 fp8 DoubleRow matmuls, one per site PAIR. lhsT [74, 2, 128] is
    block-diagonal ([W_a | 0], [0 | W_b]) so psum [128, 512] holds both
    sites' hidden pre-activations (h on partitions 0-63 / 64-127).
  - Relu + fp32->fp8 cast evacuates psum to SBUF [128, 512] tiles,
    alternating between the ACT and DVE engines.
  - L2: 25 plain fp8 matmuls accumulate logits into ONE [50, 512] psum
    tile. All 25 stationaries are sliding windows of a single [128, 98]
    tensor with the W2 column strip at cols 48/49 (pair p reads cols
    [48-2p, 98-2p), placing its two sites at out partitions 2p, 2p+1).
  - One DVE scalar_tensor_tensor: ((logit > -b2) != target) summed over
    batch -> counts [50, 1]; ones-matmul reduces over partitions; ACT
    scales by 1/(B*S). Host sums the 8 per-core partial means.
"""

import sys

for _p in ("/opt/trn_rl_repo",):
    if _p not in sys.path:
        sys.path.insert(0, _p)

import numpy as np
import ml_dtypes

B, K, H, W = 4096, 8, 64, 64
S, HID = 50, 64
NPAIR = S // 2
R = 80                 # 72 ctx rows + ones row + 7 zero pad rows: DMAs
                       # with partition counts that are multiples of 16
                       # spread across all 16 SDMA engines; 72-74 do not
N_CORES = 8
B_LOC = B // N_CORES

# ctx DMA chunk boundaries, in pairs (first small so L1_0 starts early)
CHUNKS = [(0, 2), (2, 9), (9, 17), (17, 25)]

FP8 = ml_dtypes.float8_e4m3

LAST_RESULTS = None  # test harness introspection


def _host_prep(z, W1, b1, W2, b2, b_idx, i_idx, j_idx):
    """Shard + lay out inputs; returns (in_maps, -b2)."""
    z = np.asarray(z, dtype=np.float32)
    W1 = np.asarray(W1, dtype=np.float32)
    b1 = np.asarray(b1, dtype=np.float32)
    W2 = np.asarray(W2, dtype=np.float32)
    b_idx = np.asarray(b_idx).astype(np.int64)
    i_idx = np.asarray(i_idx).astype(np.int64)
    j_idx = np.asarray(j_idx).astype(np.int64)

    di = np.repeat(np.array([-1, 0, 1]), 3)
    dj = np.tile(np.array([-1, 0, 1]), 3)
    ni = (i_idx[:, None] + di[None, :]) % H          # [S, 9]
    nj = (j_idx[:, None] + dj[None, :]) % W          # [S, 9]

    # [B, K, S, 9] -> ctx entries position-major, bit-minor: [B, S, 72]
    neigh = z[:, :, ni, nj]
    ctx_full = np.transpose(neigh, (0, 2, 3, 1)).reshape(B, S, 9 * K)
    ctx8 = ctx_full.astype(FP8)

    # per-variant [74, HID] weight matrix: entry t row = W1[t - (t > drop)],
    # row drop = 0 (the removed center-self entry), row 72 = b1, row 73 = 0.
    t = np.arange(9 * K)
    WV = np.zeros((K, R, HID), dtype=np.float32)
    for v in range(K):
        drop = 4 * K + v
        src = t - (t > drop)
        WV[v, :9 * K] = W1[np.minimum(src, 9 * K - 2)]
        WV[v, drop] = 0.0
        WV[v, 9 * K] = b1
    WV8 = WV.astype(FP8)

    # L1 stationary: [74, 50, 128]; pair p half q feeds psum partitions
    # q*64:(q+1)*64 with site 2p+q's variant matrix (block-diagonal).
    w1_np = np.zeros((R, S, 2 * HID), dtype=FP8)
    for s in range(S):
        q = s % 2
        w1_np[:, s, q * HID:(q + 1) * HID] = WV8[b_idx[s]]

    # L2 DoubleRow stationary [128, 24, 128]: group g = pairs (2g, 2g+1) =
    # sites 4g..4g+3; half q covers pair 2g+q; cols padded to 128 (DR needs
    # col_grp == 0xf). Final odd pair uses a plain [128, 50] stationary.
    W28 = W2.astype(FP8)
    w2dr_np = np.zeros((2 * HID, 2 * (NPAIR // 2), 2 * HID), dtype=FP8)
    for g in range(NPAIR // 2):
        for q in range(2):
            w2dr_np[0:HID, 2 * g + q, 4 * g + 2 * q] = W28
            w2dr_np[HID:2 * HID, 2 * g + q, 4 * g + 2 * q + 1] = W28
    w2last_np = np.zeros((2 * HID, S), dtype=FP8)
    w2last_np[0:HID, S - 2] = W28
    w2last_np[HID:2 * HID, S - 1] = W28

    in_maps = []
    for c in range(N_CORES):
        bs = slice(c * B_LOC, (c + 1) * B_LOC)
        # ctx [74, 50, 512]: [entry, site, batch]
        ctx_c = np.zeros((R, S, B_LOC), dtype=FP8)
        ctx_c[:9 * K] = ctx8[bs].transpose(2, 1, 0)
        ctx_c[9 * K] = np.float32(1.0)
        targ_c = np.ascontiguousarray(
            z[bs, b_idx, i_idx, j_idx].T)             # [50, 512] fp32
        m = {"w1a": np.ascontiguousarray(w1_np[:, 0:8, :]),
             "w1b": np.ascontiguousarray(w1_np[:, 8:S, :]),
             "w2dr": w2dr_np, "w2last": w2last_np, "targ": targ_c}
        for ci, (p0, p1) in enumerate(CHUNKS):
            m[f"ctx{ci}"] = np.ascontiguousarray(ctx_c[:, 2 * p0:2 * p1, :])
        in_maps.append(m)
    return in_maps, -float(np.asarray(b2))


def _build_program(neg_b2):
    """Emit the per-core Bass program (identical across cores)."""
    import concourse.bacc as bacc
    import concourse.mybir as mybir
    import concourse.tile as tile

    fp32 = mybir.dt.float32
    fp8 = mybir.dt.float8e4
    DR = mybir.MatmulPerfMode.DoubleRow

    nc = bacc.Bacc("TRN2", target_bir_lowering=False, debug=False,
                   num_devices=N_CORES)

    ctx_d = []
    for ci, (p0, p1) in enumerate(CHUNKS):
        ctx_d.append(nc.dram_tensor(f"ctx{ci}", [R, 2 * (p1 - p0), B_LOC],
                                    fp8, kind="ExternalInput"))
    w1a_d = nc.dram_tensor("w1a", [R, 8, 2 * HID], fp8, kind="ExternalInput")
    w1b_d = nc.dram_tensor("w1b", [R, S - 8, 2 * HID], fp8,
                           kind="ExternalInput")
    w2dr_d = nc.dram_tensor("w2dr", [2 * HID, 2 * (NPAIR // 2), 2 * HID],
                            fp8, kind="ExternalInput")
    w2last_d = nc.dram_tensor("w2last", [2 * HID, S], fp8,
                              kind="ExternalInput")
    targ_d = nc.dram_tensor("targ", [S, B_LOC], fp32, kind="ExternalInput")
    outp = nc.dram_tensor("out", [S, 1], fp32, kind="ExternalOutput")

    with tile.TileContext(nc) as tc:
        with (
            tc.tile_pool(name="const", bufs=1) as cpool,
            tc.tile_pool(name="hsb", bufs=8) as hpool,
            tc.tile_pool(name="ps", bufs=6, space="PSUM") as pspool,
            tc.tile_pool(name="psl", bufs=1, space="PSUM") as pslpool,
        ):
            # --- input DMAs -------------------------------------------
            # gpsimd (SWDGE): ctx chunks; sync/scalar HWDGE: the rest
            # (their descriptor generation runs on separate sequencers).
            ctx_t = []
            for ci, (p0, p1) in enumerate(CHUNKS):
                ct = cpool.tile([R, 2 * (p1 - p0), B_LOC], fp8,
                                tag=f"ctx{ci}", name=f"ctx_sb{ci}")
                ctx_t.append((ct, p0))
                nc.gpsimd.dma_start(out=ct[:, :, :].opt(), in_=ctx_d[ci][:, :, :].opt())

            w1_sb = cpool.tile([R, S, 2 * HID], fp8, tag="w1")
            nc.sync.dma_start(out=w1_sb[:, 0:8, :].opt(),
                              in_=w1a_d[:, :, :].opt())
            nc.sync.dma_start(out=w1_sb[:, 8:S, :].opt(),
                              in_=w1b_d[:, :, :].opt())
            w2dr_sb = cpool.tile([2 * HID, 2 * (NPAIR // 2), 2 * HID], fp8,
                                 tag="w2dr")
            nc.scalar.dma_start(out=w2dr_sb[:, :, :].opt(),
                                in_=w2dr_d[:, :, :].opt())
            w2last_sb = cpool.tile([2 * HID, S], fp8, tag="w2last")
            nc.scalar.dma_start(out=w2last_sb[:, :], in_=w2last_d[:, :])
            targ_sb = cpool.tile([S, B_LOC], fp32, tag="targ")
            nc.scalar.dma_start(out=targ_sb[:, :], in_=targ_d[:, :])

            logit_ps = pslpool.tile([2 * HID, B_LOC], fp32, tag="logit")

            # --- pair pipeline ----------------------------------------
            def pair_rhs(p):
                for ct, p0 in reversed(ctx_t):
                    if p >= p0:
                        i = p - p0
                        return ct[:, 2 * i:2 * i + 2, :]
                raise AssertionError

            h_sb = {}

            def emit_l1(p):
                h_ps = pspool.tile([2 * HID, B_LOC], fp32, tag="hps",
                                   name=f"hps{p}")
                nc.tensor.matmul(
                    h_ps[:, :],
                    w1_sb[:, 2 * p:2 * p + 2, :],
                    pair_rhs(p),
                    start=True, stop=True, perf_mode=DR)
                g = p // 2
                if g not in h_sb:
                    h_sb[g] = hpool.tile([2 * HID, 2, B_LOC], fp8,
                                         tag="hsb", name=f"hsb{g}")
                if p % 2 == 0:
                    nc.scalar.activation(
                        out=h_sb[g][:, 0, :], in_=h_ps[:, :],
                        func=mybir.ActivationFunctionType.Relu,
                        bias=0.0, scale=1.0)
                else:
                    nc.vector.tensor_scalar_max(h_sb[g][:, 1, :],
                                                h_ps[:, :], 0.0)

            def emit_l2(g):
                if 2 * g + 1 < NPAIR:
                    nc.tensor.matmul(
                        logit_ps[:, :],
                        w2dr_sb[:, 2 * g:2 * g + 2, :],
                        h_sb[g][:, :, :],
                        start=(g == 0), stop=False, perf_mode=DR)
                else:
                    nc.tensor.matmul(
                        logit_ps[0:S, :],
                        w2last_sb[:, :],
                        h_sb[g][:, 0, :],
                        start=False, stop=True)

            # L2 for group g (pairs 2g, 2g+1 = sites 4g..4g+3) emitted two
            # pairs after its evacuations; psum pool (bufs=6) keeps L1 ahead.
            for p in range(NPAIR):
                emit_l1(p)
                if p >= 6 and p % 2 == 0:
                    emit_l2((p - 6) // 2)
            emit_l2(NPAIR // 2 - 2)
            emit_l2(NPAIR // 2 - 1)
            emit_l2(NPAIR // 2)

            # --- compare + reduce -------------------------------------
            junk = cpool.tile([S, B_LOC], fp32, tag="junk")
            counts = cpool.tile([S, 1], fp32, tag="counts")
            nc.vector.scalar_tensor_tensor(
                out=junk[:, :], in0=logit_ps[0:S, :], scalar=neg_b2,
                in1=targ_sb[:, :],
                op0=mybir.AluOpType.is_gt, op1=mybir.AluOpType.not_equal,
                accum_out=counts[:, :])

            nc.sync.dma_start(out=outp[:, :], in_=counts[:, :])

    nc.compile()
    return nc


def kernel(**inputs):
    global LAST_RESULTS
    from concourse.bass_utils import run_bass_kernel_spmd

    in_maps, neg_b2 = _host_prep(
        inputs["z"], inputs["W1"], inputs["b1"], inputs["W2"],
        inputs["b2"], inputs["b_idx"], inputs["i_idx"], inputs["j_idx"])

    nc = _build_program(neg_b2)

    res = run_bass_kernel_spmd(nc, in_maps, list(range(N_CORES)))
    LAST_RESULTS = res
    total = 0.0
    for r in res.results:
        total += float(np.asarray(r["out"], dtype=np.float64).sum())
    return np.float32(total / float(B * S))
